# revision 17
# baseline (speedup 1.0000x reference)
"""CASSI layer kernel for Trainium2 (8 NeuronCores, Bass/Tile).

Math (matches the reference nn_CASSI_layer):
    H2[m,n,s]        = H[0,m,n,0,s]
    Y[b,m,n+l,s]    += H2[m,n,s] * x[b,m,n,l]            (shear-sum, l in [0,24))
    sigm             = sum(Y^2) / (M*W*B*10^(40/10))
    Yn               = Y + sqrt(sigm) * noise_eps         (noise_eps broadcast over s)
    X[b,m,n,l]       = sum_s H2[m,n,s] * Yn[b,m,n+l,s]
    out              = X / max(X)

Distribution: the (b, m) pairs form 4*256 = 1024 independent rows; each of the
8 cores takes 128 rows (core c: b = c//2, m in [128*(c%2), 128*(c%2)+128)),
mapped onto the 128 SBUF partitions.  Everything per-row lives along the free
dimension, so the spectral shifts are plain address offsets (always 4-byte
aligned in fp16 because the shift stride is S=22 elements).

The two global scalar couplings (sigm, max) are linearized out of the device
kernel: X = X0 + sqrt(sigm)*Xn with X0 the noise-free result (device) and
Xn[b,m,n,l] = (sum_s H2[m,n,s]) * noise_eps[b,m,n+l] (cheap host outer
product).  The device returns X0 and per-partition sum(Y^2); the host applies
sigma, the noise term, and the global max normalization.

Engine split per core: ScalarE broadcasts x columns over the s axis, VectorE
runs all fp16 multiplies/adds in the packed 2x perf mode, the s-contraction is
a 16+6 aligned fold tree, and sum(Y^2) rides the ScalarE Square activation's
accumulator.
"""

from contextlib import ExitStack

import numpy as np

import concourse.bass as bass
import concourse.bacc as bacc
import concourse.tile as tile
from concourse import mybir
from concourse.bass_utils import run_bass_kernel_spmd

B, M, L, S = 4, 256, 24, 22
W = M + L - 1  # 279
N_CORES = 8
ROWS = 128  # (b, m) rows per core
NOISE_DB = 40.0

_F32 = mybir.dt.float32
_F16 = mybir.dt.float16


def build_bass(dtype=_F16, gps_s2=0, gps_s4=0, tmp_bufs=2, rep_bufs=2, gps_indep_s4=5, gps_indep_s2=4, i2_bufs=2) -> bass.Bass:
    nc = bacc.Bacc()
    x_in = nc.declare_dram_parameter("x_in", [ROWS, M, L], dtype, isOutput=False)
    h_in = nc.declare_dram_parameter("h_in", [ROWS, M, S], dtype, isOutput=False)
    x0_out = nc.declare_dram_parameter("x0_out", [ROWS, M, L], dtype, isOutput=True)
    ss_out = nc.declare_dram_parameter("ss_out", [ROWS, 1], _F32, isOutput=True)

    add = mybir.AluOpType.add

    with tile.TileContext(nc) as tc, ExitStack() as ctx:
        main = ctx.enter_context(tc.tile_pool(name="main", bufs=1))
        reps = ctx.enter_context(tc.tile_pool(name="reps", bufs=rep_bufs))
        tmps = ctx.enter_context(tc.tile_pool(name="tmps", bufs=tmp_bufs))

        xs = main.tile([ROWS, M, L], dtype, tag="xs")
        hs = main.tile([ROWS, M, S], dtype, tag="hs")
        ys = main.tile([ROWS, W, S], dtype, tag="ys")
        ysb = main.tile([ROWS, W, S], dtype, tag="ysb")
        x0 = main.tile([ROWS, M, L], dtype, tag="x0")
        ss = main.tile([ROWS, 1], _F32, tag="ss")

        nc.sync.dma_start(out=hs, in_=h_in[:])
        nc.sync.dma_start(out=xs, in_=x_in[:])
        # ys gets a direct write for l=0 over w in [0, M); only its tail needs
        # zeroing.  ysb (the GPSIMD-side accumulator) is zeroed in full.
        nc.gpsimd.memset(ys[:, M:, :], 0.0)
        if gps_s2 or gps_indep_s2:
            fg = L - gps_s2 - gps_indep_s2
            nc.gpsimd.memset(ysb[:, 0:fg, :], 0.0)
            nc.gpsimd.memset(ysb[:, fg + M :, :], 0.0)

        def x_bcast(l: int) -> bass.AP:
            # x[:, :, l] broadcast along a trailing s axis: [ROWS, M, S]
            sl = xs[:, :, l]
            return bass.AP(
                tensor=sl.tensor, offset=sl.offset, ap=[sl.ap[0], sl.ap[1], [0, S]]
            )

        # Stage 1+2: Y[p, n+l, s] += H[p, n, s] * x[p, n, l]
        # ScalarE materializes the broadcast so VectorE's multiply keeps
        # step-1 fp16 operands (packed 2x mode).  The l-accumulation is split
        # across two buffers so VectorE and GPSIMD own independent chains.
        GPS_S2 = set(range(L - gps_s2, L)) if gps_s2 else set()
        GPS_I2 = set(range(L - gps_s2 - gps_indep_s2, L - gps_s2)) if gps_indep_s2 else set()
        i2pool = ctx.enter_context(tc.tile_pool(name="i2pool", bufs=i2_bufs)) if (gps_indep_s2 or gps_s2) else None
        first_gps = min(GPS_S2 | GPS_I2) if (GPS_S2 or GPS_I2) else None
        for l in range(L):
            on_gps = l in GPS_S2 or l in GPS_I2
            if on_gps and i2pool is not None:
                xr = i2pool.tile([ROWS, M, S], dtype, tag="gxr")
            else:
                xr = reps.tile([ROWS, M, S], dtype, tag="xr")
            nc.scalar.copy(out=xr, in_=x_bcast(l))
            if l == 0:
                nc.vector.tensor_mul(out=ys[:, 0:M, :], in0=hs, in1=xr)
            elif l == first_gps:
                # first GPSIMD l writes ysb directly (no add needed)
                nc.gpsimd.tensor_mul(out=ysb[:, l : l + M, :], in0=hs, in1=xr)
            elif on_gps:
                pool = i2pool if l in GPS_I2 else tmps
                tag = "g2tmp" if l in GPS_I2 else "tmp"
                tmp = pool.tile([ROWS, M, S], dtype, tag=tag)
                mul_eng = nc.gpsimd if l in GPS_I2 else nc.vector
                mul_eng.tensor_mul(out=tmp, in0=hs, in1=xr)
                ysl = ysb[:, l : l + M, :]
                nc.gpsimd.tensor_add(out=ysl, in0=ysl, in1=tmp)
            else:
                tmp = tmps.tile([ROWS, M, S], dtype, tag="tmp")
                nc.vector.tensor_mul(out=tmp, in0=hs, in1=xr)
                ysl = ys[:, l : l + M, :]
                nc.vector.tensor_add(out=ysl, in0=ysl, in1=tmp)
        if GPS_S2 or GPS_I2:
            # merge the two accumulators
            nc.vector.tensor_add(out=ys, in0=ys, in1=ysb)

        # Stage 3 partial: per-partition sum(Y^2) via ScalarE Square+accumulate.
        # ysb is dead after the merge, so it doubles as the Square write target.
        nc.scalar.activation(
            out=ysb, in_=ys, func=mybir.ActivationFunctionType.Square, accum_out=ss
        )
        nc.sync.dma_start(out=ss_out[:], in_=ss)

        # Stage 4: X0[p, n, l] = sum_s H[p, n, s] * Y[p, n+l, s]
        # s-contraction as an aligned fold tree: 22 -> 16 -> 8 -> 4 -> 2 -> 1
        # VectorE does all multiplies; fold chains are split VectorE/GPSIMD.
        FOLDS = ((0, 16, 6), (0, 8, 8), (0, 4, 4), (0, 2, 2))
        GPS_I4 = set(range(L - gps_indep_s4, L)) if gps_indep_s4 else set()
        gpool = ctx.enter_context(tc.tile_pool(name="gpool", bufs=2)) if (gps_indep_s4 or gps_indep_s2) else None
        dve_ls = [l for l in range(L) if l not in GPS_I4]

        def ap3(t, pair_step, pairs, d1_step, d1_n, d2_step, d2_n, off):
            return bass.AP(
                tensor=t.tensor,
                offset=t.offset + off,
                ap=[t.ap[0], [pair_step, pairs], [d1_step, d1_n], [d2_step, d2_n]],
            )

        # VectorE side: pair-batched pipelines (one mul + one fold tree per
        # two l values, strided across the pair axis of a double-wide tile).
        i = 0
        while i < len(dve_ls):
            l = dve_ls[i]
            if i + 1 < len(dve_ls) and dve_ls[i + 1] == l + 1:
                npair = 2
                i += 2
            else:
                npair = 1
                i += 1
            tmp = tmps.tile([ROWS, npair, M, S], dtype, tag="tmp")
            nc.vector.tensor_mul(
                out=tmp,
                in0=ap3(hs, 0, npair, S, M, 1, S, 0),
                in1=ap3(ys, S, npair, S, M, 1, S, l * S),
            )
            for dst, src, width in FOLDS:
                o = ap3(tmp, M * S, npair, S, M, 1, width, dst)
                nc.vector.tensor_tensor(
                    out=o,
                    in0=o,
                    in1=ap3(tmp, M * S, npair, S, M, 1, width, src),
                    op=add,
                )
            nc.vector.tensor_tensor(
                out=bass.AP(
                    tensor=x0.tensor,
                    offset=x0.offset + l,
                    ap=[x0.ap[0], [1, npair], [L, M]],
                ),
                in0=ap3(tmp, M * S, npair, S, M, 1, 1, 0)[:, :, :, 0],
                in1=ap3(tmp, M * S, npair, S, M, 1, 1, 1)[:, :, :, 0],
                op=add,
            )
        # GPSIMD side: independent single-l pipelines.
        for l in sorted(GPS_I4):
            tmp = gpool.tile([ROWS, M, S], dtype, tag="gtmp")
            nc.gpsimd.tensor_mul(out=tmp, in0=hs, in1=ys[:, l : l + M, :])
            for dst, src, width in FOLDS:
                o = tmp[:, :, dst : dst + width]
                nc.gpsimd.tensor_tensor(
                    out=o, in0=o, in1=tmp[:, :, src : src + width], op=add
                )
            nc.gpsimd.tensor_tensor(
                out=x0[:, :, l], in0=tmp[:, :, 0], in1=tmp[:, :, 1], op=add
            )
        nc.sync.dma_start(out=x0_out[:], in_=x0)

    nc.finalize()
    return nc


def shard_inputs(
    x: np.ndarray, H: np.ndarray, np_dtype=np.float16
) -> list[dict[str, np.ndarray]]:
    H2 = H[0, :, :, 0, :]  # (M, M, S)
    x = x.astype(np_dtype)
    H2 = H2.astype(np_dtype)
    in_maps = []
    for c in range(N_CORES):
        b, half = c // 2, c % 2
        m0 = half * ROWS
        in_maps.append(
            {
                "x_in": np.ascontiguousarray(x[b, m0 : m0 + ROWS]),
                "h_in": np.ascontiguousarray(H2[m0 : m0 + ROWS]),
            }
        )
    return in_maps


def finalize(
    results: list[dict[str, np.ndarray]],
    H: np.ndarray,
    noise_eps: np.ndarray,
) -> np.ndarray:
    X0 = np.empty((B, M, M, L), np.float32)
    sumsq = 0.0
    for c in range(N_CORES):
        b, half = c // 2, c % 2
        m0 = half * ROWS
        X0[b, m0 : m0 + ROWS] = results[c]["x0_out"].astype(np.float32)
        sumsq += results[c]["ss_out"].sum(dtype=np.float64)
    sigm = sumsq / (M * W * B * 10.0 ** (NOISE_DB / 10.0))

    H2 = H[0, :, :, 0, :]  # (M, M, S)
    hsum = H2.sum(axis=-1)  # (M, M)
    # noise window: nwin[b, m, n, l] = noise_eps[b, m, n + l, 0]
    nwin = np.lib.stride_tricks.sliding_window_view(noise_eps[:, :, :, 0], L, axis=2)
    X = X0 + np.float32(np.sqrt(sigm)) * (hsum[None, :, :, None] * nwin)
    X = X.astype(np.float32, copy=False)
    return X / X.max()


_NC_CACHE: bass.Bass | None = None


def kernel(x: np.ndarray, H: np.ndarray, noise_eps: np.ndarray) -> np.ndarray:
    global _NC_CACHE
    x = np.asarray(x, dtype=np.float32)
    H = np.asarray(H, dtype=np.float32)
    noise_eps = np.asarray(noise_eps, dtype=np.float32)
    if _NC_CACHE is None:
        _NC_CACHE = build_bass()
    in_maps = shard_inputs(x, H)
    res = run_bass_kernel_spmd(_NC_CACHE, in_maps, core_ids=list(range(N_CORES)))
    return finalize(res.results, H, noise_eps)
